# revision 14
# baseline (speedup 1.0000x reference)
"""Trainium2 Bass kernel for nn_Coembedding (dual-MLP cosine-similarity retrieval).

Computation (see reference):
    mp = relu(molecule @ Wm1.T + bm1) @ Wm2.T + bm2          [N, D]
    pp = relu(protein  @ Wp1.T + bp1) @ Wp2.T + bp2          [M, D]
    out = (pp/|pp| @ (mp/|mp|).T) / temperature              [M, N]

Distribution over 8 NeuronCores:
  - molecule rows (N) sharded 8x for the molecule MLP; normalized embeddings
    (feature-major [D, N/8] bf16) AllGathered so every core holds all N
    molecule embeddings.
  - protein rows (M) sharded 8x; each core computes its own protein MLP shard
    and the [M/8, N] similarity tile.

Perf-relevant structure (vs the fp32 baseline):
  - everything bf16 on-chip (weights, activations, similarity operands):
    halves DMA bytes, enables FWL weight loads, halves SBUF pressure.
  - whole-matrix weight DMAs (one or two descriptor-lean transfers per
    weight) spread across the sync/scalar/vector trigger engines so the
    first matmul isn't starved behind fat input transfers.
  - row norms built from scalar-engine squares + vector adds (both idle
    engines) with a single reduce-matmul; protein norms produced directly
    in column ([128, DC]) form via per-block ones-matmuls - no transpose.
  - similarity keeps all 8 gathered molecule blocks resident in SBUF and,
    for each protein 128-row tile, reuses each stationary operand across 4
    molecule blocks (two ping-ponging groups of 4 PSUM banks).
"""

import numpy as np
import ml_dtypes
from contextlib import ExitStack

import concourse.bass as bass
import concourse.tile as tile
from concourse import bacc, mybir
from concourse.bass_utils import run_bass_kernel_spmd

F32 = mybir.dt.float32
F32R = mybir.dt.float32r
BF16 = mybir.dt.bfloat16
AF = mybir.ActivationFunctionType
BF16NP = ml_dtypes.bfloat16

N_CORES = 8
N, M, MOL, PROT, D = 4096, 8192, 768, 1280, 1024
NS = N // N_CORES            # 512 molecule rows per core
MS = M // N_CORES            # 1024 protein rows per core
KM, KP, KD = MOL // 128, PROT // 128, D // 128   # 6, 10, 8 contraction chunks
DC = D // 128                # 8 output-feature chunks
EPS = 1e-8

_CACHE: dict = {}


def _build():
    if "nc" in _CACHE:
        return _CACHE["nc"]

    nc = bacc.Bacc("TRN2", target_bir_lowering=False, debug=False,
                   num_devices=N_CORES)

    # All inputs pre-tiled host-side; every DMA below is partition-major with
    # large contiguous per-partition runs (one or two DMAs per tensor).
    molT = nc.dram_tensor("molT", [128, KM, NS], BF16, kind="ExternalInput").ap()
    protT = nc.dram_tensor("protT", [128, KP, MS], BF16, kind="ExternalInput").ap()
    wm1 = nc.dram_tensor("wm1", [128, DC, KM * 128], BF16, kind="ExternalInput").ap()
    wm2 = nc.dram_tensor("wm2", [128, DC, KD * 128], BF16, kind="ExternalInput").ap()
    wp1 = nc.dram_tensor("wp1", [128, DC, KP * 128], BF16, kind="ExternalInput").ap()
    wp2 = nc.dram_tensor("wp2", [128, DC, KD * 128], BF16, kind="ExternalInput").ap()
    biases = nc.dram_tensor("biases", [128, 4, DC], F32, kind="ExternalInput").ap()
    invtemp = nc.dram_tensor("invtemp", [1, 1], F32, kind="ExternalInput").ap()
    S = nc.dram_tensor("S", [N_CORES, DC, 128, NS], F32, kind="ExternalOutput").ap()

    with tile.TileContext(nc) as tc, ExitStack() as ctx, \
            nc.allow_low_precision(reason="bf16 operands stay within the 2e-2 gate"):
        dram = ctx.enter_context(tc.tile_pool(name="dram", bufs=1, space="DRAM"))
        send = dram.tile([128, DC, NS], BF16)
        recv = dram.tile([N_CORES, 128, DC, NS], BF16, addr_space="Shared")

        sb = ctx.enter_context(tc.tile_pool(name="sb", bufs=1))
        ps = ctx.enter_context(tc.tile_pool(name="ps", bufs=8, space="PSUM"))

        # ---- constants ----
        ones_col = sb.tile([128, 1], F32, tag="onec")
        nc.gpsimd.memset(ones_col[:], 1.0)
        ones_row = sb.tile([1, 128], F32, tag="oner")
        nc.gpsimd.memset(ones_row[:], 1.0)
        invt = sb.tile([128, 1], F32, tag="invt")
        nc.gpsimd.dma_start(out=invt[:], in_=invtemp.to_broadcast([128, 1]))
        b_s = sb.tile([128, 4, DC], F32, tag="bias")
        nc.gpsimd.dma_start(out=b_s[:], in_=biases[:])

        # ---- weight / input loads (descriptor-lean, spread over queues) ----
        # Weight SBUF tiles are flat [128, DC*K]: (h, k) stationary slice at
        # offset (h*kchunks + k)*128, and every DMA is a single contiguous
        # per-partition run.
        w1 = sb.tile([128, DC * KP * 128], BF16, tag="w1")    # sized for wp1
        wm1f = wm1.rearrange("p h k -> p (h k)")
        q = DC * KM * 128 // 4
        xm = sb.tile([128, KM, NS], BF16, tag="xm")
        nc.sync.dma_start(out=xm[:, 0:KM // 2, :], in_=molT[:, 0:KM // 2, :])
        nc.scalar.dma_start(out=xm[:, KM // 2:, :], in_=molT[:, KM // 2:, :])
        nc.sync.dma_start(out=w1[:, 0:q], in_=wm1f[:, 0:q])
        nc.scalar.dma_start(out=w1[:, q:2 * q], in_=wm1f[:, q:2 * q])
        nc.sync.dma_start(out=w1[:, 2 * q:3 * q], in_=wm1f[:, 2 * q:3 * q])
        nc.scalar.dma_start(out=w1[:, 3 * q:4 * q], in_=wm1f[:, 3 * q:4 * q])
        w2 = sb.tile([128, DC * KD * 128], BF16, tag="w2")
        nc.sync.dma_start(out=w2[:], in_=wm2.rearrange("p h k -> p (h k)")[:])

        def mlp_layer(x, w, kchunks, ncols, bias, relu, out):
            """out[128, DC, ncols] (bf16) = act(w.T @ x + b); feature-major.
            w slice for (h, k) is the stationary operand; it is shared by the
            ncols//512 moving halves so LDWEIGHTS stays amortized."""
            nhalves = ncols // 512
            for h in range(DC):
                pts = [ps.tile([128, 512], F32, tag="mm", name=f"pt{_i}") for _i in range(nhalves)]
                for k in range(kchunks):
                    o = (h * kchunks + k) * 128
                    lw = w[:, o:o + 128]
                    for nh in range(nhalves):
                        nc.tensor.matmul(
                            pts[nh][:], lw,
                            x[:, k, nh * 512:(nh + 1) * 512],
                            start=(k == 0), stop=(k == kchunks - 1),
                        )
                for nh in range(nhalves):
                    nc.scalar.activation(
                        out[:, h, nh * 512:(nh + 1) * 512], pts[nh][:],
                        AF.Relu if relu else AF.Identity,
                        bias=bias[:, h:h + 1],
                    )

        # ================= molecule MLP (N shard) =================
        hm = sb.tile([128, DC, MS], BF16, tag="hid")          # sized for Hp
        mlp_layer(xm, w1, KM, NS, b_s[:, 0, :], True, hm)
        mp = sb.tile([128, DC, MS], BF16, tag="emb")          # sized for Ppb
        mlp_layer(hm, w2, KD, NS, b_s[:, 1, :], False, mp)

        # ---- molecule norms: squares on scalar, adds on vector, 1 reduce MM
        sqa = sb.tile([128, MS], F32, tag="sqa")             # sized for protein
        for k in range(DC):
            if k == 0:
                nc.scalar.square(sqa[:, 0:NS], mp[:, 0, 0:NS])
            else:
                sq = sb.tile([128, MS], F32, tag="sqt", bufs=2)
                nc.scalar.square(sq[:, 0:NS], mp[:, k, 0:NS])
                nc.vector.tensor_add(sqa[:, 0:NS], sqa[:, 0:NS], sq[:, 0:NS])
        pn = ps.tile([1, NS], F32, tag="mm")
        nc.tensor.matmul(pn[:], ones_col[:], sqa[:, 0:NS], start=True, stop=True)
        nsq = sb.tile([1, NS], F32, tag="nsq")
        nc.scalar.activation(nsq[:], pn[:], AF.Sqrt)
        nc.vector.tensor_scalar_max(nsq[:], nsq[:], EPS)
        inv = sb.tile([1, NS], F32, tag="invn")
        nc.vector.reciprocal(inv[:], nsq[:])
        pb = ps.tile([128, NS], F32, tag="mm")
        nc.tensor.matmul(pb[:], ones_row[:], inv[:], start=True, stop=True)
        binv = sb.tile([128, NS], F32, tag="binv")
        nc.scalar.activation(binv[:], pb[:], AF.Copy)
        mnb = sb.tile([128, DC, NS], BF16, tag="mnb")
        for k in range(DC):
            nc.vector.tensor_mul(mnb[:, k, :], mp[:, k, 0:NS], binv[:])
        nc.gpsimd.dma_start(out=send[:], in_=mnb[:])

        # ================= AllGather molecule embeddings =================
        nc.gpsimd.collective_compute(
            "AllGather",
            mybir.AluOpType.bypass,
            replica_groups=[list(range(N_CORES))],
            ins=[send[:]],
            outs=[recv[:]],
        )
        # prefetch the gathered blocks as soon as the collective lands (sync
        # queue is idle by now); c0..3 are needed first.
        mn_cs = []
        for c in range(N_CORES):
            t = sb.tile([128, DC, NS], BF16, tag=f"mn{c}", name=f"mn{c}")
            nc.sync.dma_start(out=t[:], in_=recv[c])
            mn_cs.append(t)

        # ================= protein MLP (M shard) =================
        xp = sb.tile([128, KP, MS], BF16, tag="xp")
        nc.scalar.dma_start(out=xp[:], in_=protT[:])
        # wp1 in four h-slabs: the tag-reuse wait releases at mol-L1 end, so
        # small slabs let protein L1 h=0 start ~8us earlier than one 2.6MB load.
        wq1 = sb.tile([128, DC * KP * 128], BF16, tag="w1")
        wp1f = wp1.rearrange("p h k -> p (h k)")
        qp = DC * KP * 128 // 4
        for j in range(4):
            nc.scalar.dma_start(out=wq1[:, j * qp:(j + 1) * qp],
                                in_=wp1f[:, j * qp:(j + 1) * qp])
        wq2 = sb.tile([128, DC * KD * 128], BF16, tag="w2")
        nc.gpsimd.dma_start(out=wq2[:], in_=wp2.rearrange("p h k -> p (h k)")[:])
        hp = sb.tile([128, DC, MS], BF16, tag="hid")
        mlp_layer(xp, wq1, KP, MS, b_s[:, 2, :], True, hp)
        ppb = sb.tile([128, DC, MS], BF16, tag="emb")
        mlp_layer(hp, wq2, KD, MS, b_s[:, 3, :], False, ppb)

        # ---- protein norms, directly in column form [128, DC] ----
        sqp = sb.tile([128, MS], F32, tag="sqa")
        for k in range(DC):
            if k == 0:
                nc.scalar.square(sqp[:], ppb[:, 0, :])
            else:
                sq = sb.tile([128, MS], F32, tag="sqt", bufs=2)
                nc.scalar.square(sq[:], ppb[:, k, :])
                nc.vector.tensor_add(sqp[:], sqp[:], sq[:])
        pcol = ps.tile([128, DC], F32, tag="mm")
        for h in range(DC):
            nc.tensor.matmul(pcol[:, h:h + 1], sqp[:, h * 128:(h + 1) * 128],
                             ones_col[:], start=(h == 0), stop=(h == DC - 1))
        ncol = sb.tile([128, DC], F32, tag="ncol")
        nc.scalar.activation(ncol[:], pcol[:], AF.Sqrt)
        nc.vector.tensor_scalar_max(ncol[:], ncol[:], EPS)
        scale_col = sb.tile([128, DC], F32, tag="scol")
        nc.vector.reciprocal(scale_col[:], ncol[:])
        nc.vector.tensor_scalar_mul(scale_col[:], scale_col[:], invt[:, 0:1])

        # ================= similarity tiles =================
        # For each 128-protein tile mi, each stationary operand ppb[:,k,mi]
        # serves 4 molecule blocks; the two 4-bank groups ping-pong so one
        # group's eviction overlaps the other's accumulation.
        for mi in range(MS // 128):
            for g in range(2):
                pts = [ps.tile([128, 512], F32, tag="mm", name=f"spt{_i}") for _i in range(4)]
                for k in range(KD):
                    lw = ppb[:, k, mi * 128:(mi + 1) * 128]
                    for ci in range(4):
                        nc.tensor.matmul(
                            pts[ci][:], lw, mn_cs[g * 4 + ci][:, k, :],
                            start=(k == 0), stop=(k == KD - 1),
                        )
                for ci in range(4):
                    stile = sb.tile([128, NS], F32, tag="st", bufs=8)
                    # alternate eviction engines so the 4 tile evictions of a
                    # group drain in ~2 slots instead of serializing on scalar
                    if ci % 2 == 0:
                        nc.scalar.activation(stile[:], pts[ci][:], AF.Copy,
                                             scale=scale_col[:, mi:mi + 1])
                    else:
                        nc.vector.tensor_scalar_mul(stile[:], pts[ci][:],
                                                    scale_col[:, mi:mi + 1])
                    nc.gpsimd.dma_start(out=S[g * 4 + ci, mi], in_=stile[:])

    nc.compile()
    _CACHE["nc"] = nc
    return nc


def _tile_w(W):
    """W [Dout, K] (fp32) -> bf16 [128, DC, K] where element (p, h, k*128+m) =
    W[h*128+m, k*128+p]: whole matrix is one partition-major linear load and
    slice [:, h, k*128:(k+1)*128] is a ready-to-use stationary operand."""
    Dout, K = W.shape
    kc = K // 128
    t = W.reshape(DC, 128, kc, 128).transpose(3, 0, 2, 1)   # [p, h, k, m]
    return np.ascontiguousarray(t.reshape(128, DC, kc * 128).astype(BF16NP))


def _tile_x(Xshard):
    """X [rows, K] -> bf16 [128, KC, rows] feature-major partition-tiled."""
    rows, K = Xshard.shape
    kc = K // 128
    t = Xshard.reshape(rows, kc, 128).transpose(2, 1, 0)    # [p, k, rows]
    return np.ascontiguousarray(t.astype(BF16NP))


def kernel(molecule, protein, Wm1, bm1, Wm2, bm2, Wp1, bp1, Wp2, bp2,
           temperature):
    nc = _build()

    molecule = np.asarray(molecule, np.float32)
    protein = np.asarray(protein, np.float32)
    wm1 = _tile_w(np.asarray(Wm1, np.float32))
    wm2 = _tile_w(np.asarray(Wm2, np.float32))
    wp1 = _tile_w(np.asarray(Wp1, np.float32))
    wp2 = _tile_w(np.asarray(Wp2, np.float32))

    def tile_b(b):
        return np.asarray(b, np.float32).reshape(DC, 128).T

    biases = np.ascontiguousarray(np.stack(
        [tile_b(bm1), tile_b(bm2), tile_b(bp1), tile_b(bp2)], axis=1))
    invt = (1.0 / np.asarray(temperature, np.float32)).reshape(1, 1)

    in_maps = []
    for c in range(N_CORES):
        in_maps.append({
            "molT": _tile_x(molecule[c * NS:(c + 1) * NS]),
            "protT": _tile_x(protein[c * MS:(c + 1) * MS]),
            "wm1": wm1, "wm2": wm2, "wp1": wp1, "wp2": wp2,
            "biases": biases, "invtemp": invt,
        })

    _CACHE["in_maps"] = in_maps
    res = run_bass_kernel_spmd(nc, in_maps, list(range(N_CORES)))
    out = np.empty((M, N), np.float32)
    for c in range(N_CORES):
        # S block layout [c2, mi, 128, 512] -> rows mi*128+i, cols c2*512+j
        blk = res.results[c]["S"]                      # [8, 8, 128, 512]
        out[c * MS:(c + 1) * MS] = blk.transpose(1, 2, 0, 3).reshape(MS, N)
    return out


# revision 18
# speedup vs baseline: 1.0073x; 1.0073x over previous
"""Trainium2 Bass kernel for nn_Coembedding (dual-MLP cosine-similarity retrieval).

Computation (see reference):
    mp = relu(molecule @ Wm1.T + bm1) @ Wm2.T + bm2          [N, D]
    pp = relu(protein  @ Wp1.T + bp1) @ Wp2.T + bp2          [M, D]
    out = (pp/|pp| @ (mp/|mp|).T) / temperature              [M, N]

Distribution over 8 NeuronCores:
  - molecule rows (N) sharded 8x for the molecule MLP; normalized embeddings
    (feature-major [D, N/8] bf16) AllGathered so every core holds all N
    molecule embeddings.
  - protein rows (M) sharded 8x; each core computes its own protein MLP shard
    and the [M/8, N] similarity tile.

Perf-relevant structure (vs the fp32 baseline):
  - everything bf16 on-chip (weights, activations, similarity operands):
    halves DMA bytes, enables FWL weight loads, halves SBUF pressure.
  - whole-matrix weight DMAs (one or two descriptor-lean transfers per
    weight) spread across the sync/scalar/vector trigger engines so the
    first matmul isn't starved behind fat input transfers.
  - row norms built from scalar-engine squares + vector adds (both idle
    engines) with a single reduce-matmul; protein norms produced directly
    in column ([128, DC]) form via per-block ones-matmuls - no transpose.
  - similarity keeps all 8 gathered molecule blocks resident in SBUF and,
    for each protein 128-row tile, reuses each stationary operand across 4
    molecule blocks (two ping-ponging groups of 4 PSUM banks).
"""

import numpy as np
import ml_dtypes
from contextlib import ExitStack

import concourse.bass as bass
import concourse.tile as tile
from concourse import bacc, mybir
from concourse.bass_utils import run_bass_kernel_spmd

F32 = mybir.dt.float32
F32R = mybir.dt.float32r
BF16 = mybir.dt.bfloat16
AF = mybir.ActivationFunctionType
BF16NP = ml_dtypes.bfloat16

N_CORES = 8
N, M, MOL, PROT, D = 4096, 8192, 768, 1280, 1024
NS = N // N_CORES            # 512 molecule rows per core
MS = M // N_CORES            # 1024 protein rows per core
KM, KP, KD = MOL // 128, PROT // 128, D // 128   # 6, 10, 8 contraction chunks
DC = D // 128                # 8 output-feature chunks
EPS = 1e-8

_CACHE: dict = {}


def _build():
    if "nc" in _CACHE:
        return _CACHE["nc"]

    nc = bacc.Bacc("TRN2", target_bir_lowering=False, debug=False,
                   num_devices=N_CORES)

    # All inputs pre-tiled host-side; every DMA below is partition-major with
    # large contiguous per-partition runs (one or two DMAs per tensor).
    molT = nc.dram_tensor("molT", [128, KM, NS], BF16, kind="ExternalInput").ap()
    protT = nc.dram_tensor("protT", [128, KP, MS], BF16, kind="ExternalInput").ap()
    wm1 = nc.dram_tensor("wm1", [128, DC, KM * 128], BF16, kind="ExternalInput").ap()
    wm2 = nc.dram_tensor("wm2", [128, DC, KD * 128], BF16, kind="ExternalInput").ap()
    wp1 = nc.dram_tensor("wp1", [128, DC, KP * 128], BF16, kind="ExternalInput").ap()
    wp2 = nc.dram_tensor("wp2", [128, DC, KD * 128], BF16, kind="ExternalInput").ap()
    biases = nc.dram_tensor("biases", [128, 4, DC], F32, kind="ExternalInput").ap()
    invtemp = nc.dram_tensor("invtemp", [1, 1], F32, kind="ExternalInput").ap()
    S = nc.dram_tensor("S", [N_CORES, DC, 128, NS], F32, kind="ExternalOutput").ap()

    with tile.TileContext(nc) as tc, ExitStack() as ctx, \
            nc.allow_low_precision(reason="bf16 operands stay within the 2e-2 gate"):
        dram = ctx.enter_context(tc.tile_pool(name="dram", bufs=1, space="DRAM"))
        # raw molecule embeddings + a packed [1, NS] inverse-norm sidecar row
        send = dram.tile([128, DC * NS + NS], BF16)
        recv = dram.tile([N_CORES, 128, DC * NS + NS], BF16, addr_space="Shared")

        sb = ctx.enter_context(tc.tile_pool(name="sb", bufs=1))
        ps = ctx.enter_context(tc.tile_pool(name="ps", bufs=8, space="PSUM"))

        # ---- constants ----
        ones_col = sb.tile([128, 1], F32, tag="onec")
        nc.gpsimd.memset(ones_col[:], 1.0)
        ones_row = sb.tile([1, 128], BF16, tag="oner")
        nc.gpsimd.memset(ones_row[:], 1.0)
        invt = sb.tile([128, 1], F32, tag="invt")
        nc.gpsimd.dma_start(out=invt[:], in_=invtemp.to_broadcast([128, 1]))
        b_s = sb.tile([128, 4, DC], F32, tag="bias")
        nc.gpsimd.dma_start(out=b_s[:], in_=biases[:])

        # ---- weight / input loads (descriptor-lean, spread over queues) ----
        # Weight SBUF tiles are flat [128, DC*K]: (h, k) stationary slice at
        # offset (h*kchunks + k)*128, and every DMA is a single contiguous
        # per-partition run.
        w1 = sb.tile([128, DC * KP * 128], BF16, tag="w1")    # sized for wp1
        wm1f = wm1.rearrange("p h k -> p (h k)")
        q = DC * KM * 128 // 4
        xm = sb.tile([128, KM, NS], BF16, tag="xm")
        nc.sync.dma_start(out=xm[:, 0:KM // 2, :], in_=molT[:, 0:KM // 2, :])
        nc.scalar.dma_start(out=xm[:, KM // 2:, :], in_=molT[:, KM // 2:, :])
        nc.sync.dma_start(out=w1[:, 0:q], in_=wm1f[:, 0:q])
        nc.scalar.dma_start(out=w1[:, q:2 * q], in_=wm1f[:, q:2 * q])
        nc.sync.dma_start(out=w1[:, 2 * q:3 * q], in_=wm1f[:, 2 * q:3 * q])
        nc.scalar.dma_start(out=w1[:, 3 * q:4 * q], in_=wm1f[:, 3 * q:4 * q])
        w2 = sb.tile([128, DC * KD * 128], BF16, tag="w2")
        nc.sync.dma_start(out=w2[:], in_=wm2.rearrange("p h k -> p (h k)")[:])

        def mlp_layer(x, w, kchunks, ncols, bias, relu, out):
            """out[128, DC, ncols] (bf16) = act(w.T @ x + b); feature-major.
            w slice for (h, k) is the stationary operand; it is shared by the
            ncols//512 moving halves so LDWEIGHTS stays amortized."""
            nhalves = ncols // 512
            for h in range(DC):
                pts = [ps.tile([128, 512], F32, tag="mm", name=f"pt{_i}") for _i in range(nhalves)]
                for k in range(kchunks):
                    o = (h * kchunks + k) * 128
                    lw = w[:, o:o + 128]
                    for nh in range(nhalves):
                        nc.tensor.matmul(
                            pts[nh][:], lw,
                            x[:, k, nh * 512:(nh + 1) * 512],
                            start=(k == 0), stop=(k == kchunks - 1),
                        )
                for nh in range(nhalves):
                    nc.scalar.activation(
                        out[:, h, nh * 512:(nh + 1) * 512], pts[nh][:],
                        AF.Relu if relu else AF.Identity,
                        bias=bias[:, h:h + 1],
                    )

        # ================= molecule MLP (N shard) =================
        hm = sb.tile([128, DC, MS], BF16, tag="hid")          # sized for Hp
        mlp_layer(xm, w1, KM, NS, b_s[:, 0, :], True, hm)
        mp = sb.tile([128, DC, NS], BF16, tag="molemb")
        mlp_layer(hm, w2, KD, NS, b_s[:, 1, :], False, mp)

        # ---- molecule norms: squares on scalar, adds on vector, 1 reduce MM
        sqa = sb.tile([128, MS], F32, tag="sqa")             # sized for protein
        for k in range(DC):
            if k == 0:
                nc.scalar.square(sqa[:, 0:NS], mp[:, 0, 0:NS])
            else:
                sq = sb.tile([128, MS], F32, tag="sqt", bufs=2)
                nc.scalar.square(sq[:, 0:NS], mp[:, k, 0:NS])
                nc.vector.tensor_add(sqa[:, 0:NS], sqa[:, 0:NS], sq[:, 0:NS])
        pn = ps.tile([1, NS], F32, tag="mm")
        nc.tensor.matmul(pn[:], ones_col[:], sqa[:, 0:NS], start=True, stop=True)
        nsq = sb.tile([1, NS], F32, tag="nsq")
        nc.scalar.activation(nsq[:], pn[:], AF.Sqrt)
        nc.vector.tensor_scalar_max(nsq[:], nsq[:], EPS)
        inv = sb.tile([1, NS], F32, tag="invn")
        nc.vector.reciprocal(inv[:], nsq[:])
        inv_b = sb.tile([1, NS], BF16, tag="invb")
        nc.vector.tensor_copy(inv_b[:], inv[:])
        # send the RAW embeddings plus the inverse norms; normalization is
        # applied on the receive side (folded into the S-tile eviction), so
        # the AllGather issues ~5us earlier.
        nc.gpsimd.dma_start(out=send[:, 0:DC * NS],
                            in_=mp.rearrange("p h n -> p (h n)")[:])
        nc.gpsimd.dma_start(out=send[0:1, DC * NS:], in_=inv_b[:])

        # ================= AllGather molecule embeddings =================
        nc.gpsimd.collective_compute(
            "AllGather",
            mybir.AluOpType.bypass,
            replica_groups=[list(range(N_CORES))],
            ins=[send[:]],
            outs=[recv[:]],
        )
        # ================= protein MLP (M shard) =================
        xp = sb.tile([128, KP, MS], BF16, tag="xp")
        nc.scalar.dma_start(out=xp[:], in_=protT[:])
        # wp1 in four h-slabs: the tag-reuse wait releases at mol-L1 end, so
        # small slabs let protein L1 h=0 start ~8us earlier than one 2.6MB load.
        wq1 = sb.tile([128, DC * KP * 128], BF16, tag="w1")
        wp1f = wp1.rearrange("p h k -> p (h k)")
        qp = DC * KP * 128 // 4
        for j in range(4):
            nc.scalar.dma_start(out=wq1[:, j * qp:(j + 1) * qp],
                                in_=wp1f[:, j * qp:(j + 1) * qp])
        wq2 = sb.tile([128, DC * KD * 128], BF16, tag="w2")
        nc.gpsimd.dma_start(out=wq2[:], in_=wp2.rearrange("p h k -> p (h k)")[:])
        hp = sb.tile([128, DC, MS], BF16, tag="hid")
        mlp_layer(xp, wq1, KP, MS, b_s[:, 2, :], True, hp)
        ppb = sb.tile([128, DC, MS], BF16, tag="emb")
        mlp_layer(hp, wq2, KD, MS, b_s[:, 3, :], False, ppb)

        # ---- post-AG prefetch: blocks on the idle sync ring, inverse-norm
        # rows on gpsimd. Placed after the protein MLP in program order so no
        # engine with pending compute sits on the collective wait.
        mn_cs = []
        inv_cs = []
        for c in range(N_CORES):
            t = sb.tile([128, DC * NS], BF16, tag=f"mn{c}", name=f"mn{c}")
            nc.sync.dma_start(out=t[:], in_=recv[c][:, 0:DC * NS])
            mn_cs.append(t)
            ic = sb.tile([1, NS], BF16, tag=f"ic{c}", name=f"ic{c}")
            nc.gpsimd.dma_start(out=ic[:], in_=recv[c][0:1, DC * NS:])
            inv_cs.append(ic)

        # ---- protein norms, directly in column form [128, DC] ----
        sqp = sb.tile([128, MS], F32, tag="sqa")
        for k in range(DC):
            if k == 0:
                nc.scalar.square(sqp[:], ppb[:, 0, :])
            else:
                sq = sb.tile([128, MS], F32, tag="sqt", bufs=2)
                nc.scalar.square(sq[:], ppb[:, k, :])
                nc.vector.tensor_add(sqp[:], sqp[:], sq[:])
        pcol = ps.tile([128, DC], F32, tag="mm")
        for h in range(DC):
            nc.tensor.matmul(pcol[:, h:h + 1], sqp[:, h * 128:(h + 1) * 128],
                             ones_col[:], start=(h == 0), stop=(h == DC - 1))
        ncol = sb.tile([128, DC], F32, tag="ncol")
        nc.scalar.activation(ncol[:], pcol[:], AF.Sqrt)
        nc.vector.tensor_scalar_max(ncol[:], ncol[:], EPS)
        scale_col = sb.tile([128, DC], F32, tag="scol")
        nc.vector.reciprocal(scale_col[:], ncol[:])
        nc.vector.tensor_scalar_mul(scale_col[:], scale_col[:], invt[:, 0:1])

        # broadcast each block's molecule inverse norms to all partitions
        binv_cs = []
        for c in range(N_CORES):
            pbc = ps.tile([128, NS], F32, tag="mm", name=f"pbc{c}")
            nc.tensor.matmul(pbc[:], ones_row[:], inv_cs[c][:],
                             start=True, stop=True)
            bc = sb.tile([128, NS], BF16, tag=f"bc{c}", name=f"bc{c}")
            nc.scalar.activation(bc[:], pbc[:], AF.Copy)
            binv_cs.append(bc)

        # ================= similarity tiles =================
        # For each 128-protein tile mi, each stationary operand ppb[:,k,mi]
        # serves 4 molecule blocks; the two 4-bank groups ping-pong so one
        # group's eviction overlaps the other's accumulation.
        for mi in range(MS // 128):
            for g in range(2):
                pts = [ps.tile([128, 512], F32, tag="mm", name=f"spt{_i}") for _i in range(4)]
                for k in range(KD):
                    lw = ppb[:, k, mi * 128:(mi + 1) * 128]
                    for ci in range(4):
                        nc.tensor.matmul(
                            pts[ci][:], lw, mn_cs[g * 4 + ci][:, k * NS:(k + 1) * NS],
                            start=(k == 0), stop=(k == KD - 1),
                        )
                for ci in range(4):
                    stile = sb.tile([128, NS], F32, tag="st", bufs=4)
                    # (psum * protein_scale) * molecule_inv_norms in one DVE op
                    nc.vector.scalar_tensor_tensor(
                        stile[:], pts[ci][:], scale_col[:, mi:mi + 1],
                        binv_cs[g * 4 + ci][:],
                        mybir.AluOpType.mult, mybir.AluOpType.mult)
                    nc.gpsimd.dma_start(out=S[g * 4 + ci, mi], in_=stile[:])

    nc.compile()
    _CACHE["nc"] = nc
    return nc


def _tile_w(W):
    """W [Dout, K] (fp32) -> bf16 [128, DC, K] where element (p, h, k*128+m) =
    W[h*128+m, k*128+p]: whole matrix is one partition-major linear load and
    slice [:, h, k*128:(k+1)*128] is a ready-to-use stationary operand."""
    Dout, K = W.shape
    kc = K // 128
    t = W.reshape(DC, 128, kc, 128).transpose(3, 0, 2, 1)   # [p, h, k, m]
    return np.ascontiguousarray(t.reshape(128, DC, kc * 128).astype(BF16NP))


def _tile_x(Xshard):
    """X [rows, K] -> bf16 [128, KC, rows] feature-major partition-tiled."""
    rows, K = Xshard.shape
    kc = K // 128
    t = Xshard.reshape(rows, kc, 128).transpose(2, 1, 0)    # [p, k, rows]
    return np.ascontiguousarray(t.astype(BF16NP))


def kernel(molecule, protein, Wm1, bm1, Wm2, bm2, Wp1, bp1, Wp2, bp2,
           temperature):
    nc = _build()

    molecule = np.asarray(molecule, np.float32)
    protein = np.asarray(protein, np.float32)
    wm1 = _tile_w(np.asarray(Wm1, np.float32))
    wm2 = _tile_w(np.asarray(Wm2, np.float32))
    wp1 = _tile_w(np.asarray(Wp1, np.float32))
    wp2 = _tile_w(np.asarray(Wp2, np.float32))

    def tile_b(b):
        return np.asarray(b, np.float32).reshape(DC, 128).T

    biases = np.ascontiguousarray(np.stack(
        [tile_b(bm1), tile_b(bm2), tile_b(bp1), tile_b(bp2)], axis=1))
    invt = (1.0 / np.asarray(temperature, np.float32)).reshape(1, 1)

    in_maps = []
    for c in range(N_CORES):
        in_maps.append({
            "molT": _tile_x(molecule[c * NS:(c + 1) * NS]),
            "protT": _tile_x(protein[c * MS:(c + 1) * MS]),
            "wm1": wm1, "wm2": wm2, "wp1": wp1, "wp2": wp2,
            "biases": biases, "invtemp": invt,
        })

    _CACHE["in_maps"] = in_maps
    res = run_bass_kernel_spmd(nc, in_maps, list(range(N_CORES)))
    out = np.empty((M, N), np.float32)
    for c in range(N_CORES):
        # S block layout [c2, mi, 128, 512] -> rows mi*128+i, cols c2*512+j
        blk = res.results[c]["S"]                      # [8, 8, 128, 512]
        out[c * MS:(c + 1) * MS] = blk.transpose(1, 2, 0, 3).reshape(MS, N)
    return out


# revision 20
# speedup vs baseline: 1.0125x; 1.0052x over previous
"""Trainium2 Bass kernel for nn_Coembedding (dual-MLP cosine-similarity retrieval).

Computation (see reference):
    mp = relu(molecule @ Wm1.T + bm1) @ Wm2.T + bm2          [N, D]
    pp = relu(protein  @ Wp1.T + bp1) @ Wp2.T + bp2          [M, D]
    out = (pp/|pp| @ (mp/|mp|).T) / temperature              [M, N]

Distribution over 8 NeuronCores:
  - molecule rows (N) sharded 8x for the molecule MLP; normalized embeddings
    (feature-major [D, N/8] bf16) AllGathered so every core holds all N
    molecule embeddings.
  - protein rows (M) sharded 8x; each core computes its own protein MLP shard
    and the [M/8, N] similarity tile.

Perf-relevant structure (vs the fp32 baseline):
  - everything bf16 on-chip (weights, activations, similarity operands):
    halves DMA bytes, enables FWL weight loads, halves SBUF pressure.
  - whole-matrix weight DMAs (one or two descriptor-lean transfers per
    weight) spread across the sync/scalar/vector trigger engines so the
    first matmul isn't starved behind fat input transfers.
  - row norms built from scalar-engine squares + vector adds (both idle
    engines) with a single reduce-matmul; protein norms produced directly
    in column ([128, DC]) form via per-block ones-matmuls - no transpose.
  - similarity keeps all 8 gathered molecule blocks resident in SBUF and,
    for each protein 128-row tile, reuses each stationary operand across 4
    molecule blocks (two ping-ponging groups of 4 PSUM banks).
"""

import numpy as np
import ml_dtypes
from contextlib import ExitStack

import concourse.bass as bass
import concourse.tile as tile
from concourse import bacc, mybir
from concourse.bass_utils import run_bass_kernel_spmd

F32 = mybir.dt.float32
F32R = mybir.dt.float32r
BF16 = mybir.dt.bfloat16
AF = mybir.ActivationFunctionType
BF16NP = ml_dtypes.bfloat16

N_CORES = 8
N, M, MOL, PROT, D = 4096, 8192, 768, 1280, 1024
NS = N // N_CORES            # 512 molecule rows per core
MS = M // N_CORES            # 1024 protein rows per core
KM, KP, KD = MOL // 128, PROT // 128, D // 128   # 6, 10, 8 contraction chunks
DC = D // 128                # 8 output-feature chunks
EPS = 1e-8

_CACHE: dict = {}


def _build():
    if "nc" in _CACHE:
        return _CACHE["nc"]

    nc = bacc.Bacc("TRN2", target_bir_lowering=False, debug=False,
                   num_devices=N_CORES)

    # All inputs pre-tiled host-side; every DMA below is partition-major with
    # large contiguous per-partition runs (one or two DMAs per tensor).
    molT = nc.dram_tensor("molT", [128, KM, NS], BF16, kind="ExternalInput").ap()
    protT = nc.dram_tensor("protT", [128, KP, MS], BF16, kind="ExternalInput").ap()
    wm1 = nc.dram_tensor("wm1", [128, DC, KM * 128], BF16, kind="ExternalInput").ap()
    wm2 = nc.dram_tensor("wm2", [128, DC, KD * 128], BF16, kind="ExternalInput").ap()
    wp1 = nc.dram_tensor("wp1", [128, DC, KP * 128], BF16, kind="ExternalInput").ap()
    wp2 = nc.dram_tensor("wp2", [128, DC, KD * 128], BF16, kind="ExternalInput").ap()
    biases = nc.dram_tensor("biases", [128, 4, DC], F32, kind="ExternalInput").ap()
    invtemp = nc.dram_tensor("invtemp", [1, 1], F32, kind="ExternalInput").ap()
    S = nc.dram_tensor("S", [N_CORES, DC, 128, NS], F32, kind="ExternalOutput").ap()

    with tile.TileContext(nc) as tc, ExitStack() as ctx, \
            nc.allow_low_precision(reason="bf16 operands stay within the 2e-2 gate"):
        dram = ctx.enter_context(tc.tile_pool(name="dram", bufs=1, space="DRAM"))
        # raw molecule embeddings + a packed [1, NS] inverse-norm sidecar row
        send = dram.tile([128, DC * NS + NS], BF16)
        recv = dram.tile([N_CORES, 128, DC * NS + NS], BF16, addr_space="Shared")

        sb = ctx.enter_context(tc.tile_pool(name="sb", bufs=1))
        ps = ctx.enter_context(tc.tile_pool(name="ps", bufs=8, space="PSUM"))

        # ---- constants ----
        ones_col = sb.tile([128, 1], F32, tag="onec")
        nc.gpsimd.memset(ones_col[:], 1.0)
        ones_row = sb.tile([1, 128], BF16, tag="oner")
        nc.gpsimd.memset(ones_row[:], 1.0)
        invt = sb.tile([128, 1], F32, tag="invt")
        nc.gpsimd.dma_start(out=invt[:], in_=invtemp.to_broadcast([128, 1]))
        b_s = sb.tile([128, 4, DC], F32, tag="bias")
        nc.gpsimd.dma_start(out=b_s[:], in_=biases[:])

        # ---- weight / input loads (descriptor-lean, spread over queues) ----
        # Weight SBUF tiles are flat [128, DC*K]: (h, k) stationary slice at
        # offset (h*kchunks + k)*128, and every DMA is a single contiguous
        # per-partition run.
        w1 = sb.tile([128, DC * KP * 128], BF16, tag="w1")    # sized for wp1
        wm1f = wm1.rearrange("p h k -> p (h k)")
        q = DC * KM * 128 // 4
        xm = sb.tile([128, KM, NS], BF16, tag="xm")
        nc.sync.dma_start(out=xm[:, 0:KM // 2, :], in_=molT[:, 0:KM // 2, :])
        nc.scalar.dma_start(out=xm[:, KM // 2:, :], in_=molT[:, KM // 2:, :])
        nc.sync.dma_start(out=w1[:, 0:q], in_=wm1f[:, 0:q])
        nc.scalar.dma_start(out=w1[:, q:2 * q], in_=wm1f[:, q:2 * q])
        nc.sync.dma_start(out=w1[:, 2 * q:3 * q], in_=wm1f[:, 2 * q:3 * q])
        nc.scalar.dma_start(out=w1[:, 3 * q:4 * q], in_=wm1f[:, 3 * q:4 * q])
        w2 = sb.tile([128, DC * KD * 128], BF16, tag="w2")
        nc.sync.dma_start(out=w2[:], in_=wm2.rearrange("p h k -> p (h k)")[:])

        def mlp_layer(x, w, kchunks, ncols, bias, relu, out):
            """out[128, DC, ncols] (bf16) = act(w.T @ x + b); feature-major.
            w slice for (h, k) is the stationary operand; it is shared by the
            ncols//512 moving halves so LDWEIGHTS stays amortized."""
            nhalves = ncols // 512
            for h in range(DC):
                pts = [ps.tile([128, 512], F32, tag="mm", name=f"pt{_i}") for _i in range(nhalves)]
                for k in range(kchunks):
                    o = (h * kchunks + k) * 128
                    lw = w[:, o:o + 128]
                    for nh in range(nhalves):
                        nc.tensor.matmul(
                            pts[nh][:], lw,
                            x[:, k, nh * 512:(nh + 1) * 512],
                            start=(k == 0), stop=(k == kchunks - 1),
                        )
                for nh in range(nhalves):
                    nc.scalar.activation(
                        out[:, h, nh * 512:(nh + 1) * 512], pts[nh][:],
                        AF.Relu if relu else AF.Identity,
                        bias=bias[:, h:h + 1],
                    )

        # ================= molecule MLP (N shard) =================
        hm = sb.tile([128, DC, MS], BF16, tag="hid")          # sized for Hp
        mlp_layer(xm, w1, KM, NS, b_s[:, 0, :], True, hm)
        mp = sb.tile([128, DC, NS], BF16, tag="molemb")
        mlp_layer(hm, w2, KD, NS, b_s[:, 1, :], False, mp)

        # ---- molecule norms: squares on scalar, adds on vector, 1 reduce MM
        sqa = sb.tile([128, MS], F32, tag="sqa")             # sized for protein
        for k in range(DC):
            if k == 0:
                nc.scalar.square(sqa[:, 0:NS], mp[:, 0, 0:NS])
            else:
                sq = sb.tile([128, MS], F32, tag="sqt", bufs=2)
                nc.scalar.square(sq[:, 0:NS], mp[:, k, 0:NS])
                nc.vector.tensor_add(sqa[:, 0:NS], sqa[:, 0:NS], sq[:, 0:NS])
        pn = ps.tile([1, NS], F32, tag="mm")
        nc.tensor.matmul(pn[:], ones_col[:], sqa[:, 0:NS], start=True, stop=True)
        nsq = sb.tile([1, NS], F32, tag="nsq")
        nc.scalar.activation(nsq[:], pn[:], AF.Sqrt)
        nc.vector.tensor_scalar_max(nsq[:], nsq[:], EPS)
        inv = sb.tile([1, NS], F32, tag="invn")
        nc.vector.reciprocal(inv[:], nsq[:])
        inv_b = sb.tile([1, NS], BF16, tag="invb")
        nc.vector.tensor_copy(inv_b[:], inv[:])
        # send the RAW embeddings plus the inverse norms; normalization is
        # applied on the receive side (folded into the S-tile eviction), so
        # the AllGather issues ~5us earlier.
        nc.gpsimd.dma_start(out=send[:, 0:DC * NS],
                            in_=mp.rearrange("p h n -> p (h n)")[:])
        nc.gpsimd.dma_start(out=send[0:1, DC * NS:], in_=inv_b[:])

        # ================= AllGather molecule embeddings =================
        nc.gpsimd.collective_compute(
            "AllGather",
            mybir.AluOpType.bypass,
            replica_groups=[list(range(N_CORES))],
            ins=[send[:]],
            outs=[recv[:]],
        )
        # ================= protein MLP (M shard) =================
        xp = sb.tile([128, KP, MS], BF16, tag="xp")
        nc.scalar.dma_start(out=xp[:], in_=protT[:])
        # wp1 in four h-slabs: the tag-reuse wait releases at mol-L1 end, so
        # small slabs let protein L1 h=0 start ~8us earlier than one 2.6MB load.
        wq1 = sb.tile([128, DC * KP * 128], BF16, tag="w1")
        wp1f = wp1.rearrange("p h k -> p (h k)")
        qp = DC * KP * 128 // 4
        for j in range(4):
            nc.scalar.dma_start(out=wq1[:, j * qp:(j + 1) * qp],
                                in_=wp1f[:, j * qp:(j + 1) * qp])
        wq2 = sb.tile([128, DC * KD * 128], BF16, tag="w2")
        nc.gpsimd.dma_start(out=wq2[:], in_=wp2.rearrange("p h k -> p (h k)")[:])
        hp = sb.tile([128, DC, MS], BF16, tag="hid")
        mlp_layer(xp, wq1, KP, MS, b_s[:, 2, :], True, hp)
        ppb = sb.tile([128, DC, MS], BF16, tag="emb")
        mlp_layer(hp, wq2, KD, MS, b_s[:, 3, :], False, ppb)

        # ---- post-AG prefetch: blocks on the idle sync ring, inverse-norm
        # rows on gpsimd. Placed after the protein MLP in program order so no
        # engine with pending compute sits on the collective wait.
        mn_cs = []
        inv_cs = []
        for c in range(N_CORES):
            t = sb.tile([128, DC * NS], BF16, tag=f"mn{c}", name=f"mn{c}")
            nc.sync.dma_start(out=t[:], in_=recv[c][:, 0:DC * NS])
            mn_cs.append(t)
            ic = sb.tile([1, NS], BF16, tag=f"ic{c}", name=f"ic{c}")
            nc.gpsimd.dma_start(out=ic[:], in_=recv[c][0:1, DC * NS:])
            inv_cs.append(ic)

        # ---- protein norms, directly in column form [128, DC] ----
        sqp = sb.tile([128, MS], F32, tag="sqa")
        for k in range(DC):
            if k == 0:
                nc.scalar.square(sqp[:], ppb[:, 0, :])
            else:
                sq = sb.tile([128, MS], F32, tag="sqt", bufs=2)
                nc.scalar.square(sq[:], ppb[:, k, :])
                nc.vector.tensor_add(sqp[:], sqp[:], sq[:])
        pcol = ps.tile([128, DC], F32, tag="mm")
        for h in range(DC):
            nc.tensor.matmul(pcol[:, h:h + 1], sqp[:, h * 128:(h + 1) * 128],
                             ones_col[:], start=(h == 0), stop=(h == DC - 1))
        ncol = sb.tile([128, DC], F32, tag="ncol")
        nc.scalar.activation(ncol[:], pcol[:], AF.Sqrt)
        nc.vector.tensor_scalar_max(ncol[:], ncol[:], EPS)
        scale_col = sb.tile([128, DC], F32, tag="scol")
        nc.vector.reciprocal(scale_col[:], ncol[:])
        nc.vector.tensor_scalar_mul(scale_col[:], scale_col[:], invt[:, 0:1])

        # broadcast each block's molecule inverse norms to all partitions
        binv_cs = []
        for c in range(N_CORES):
            pbc = ps.tile([128, NS], F32, tag="mm", name=f"pbc{c}")
            nc.tensor.matmul(pbc[:], ones_row[:], inv_cs[c][:],
                             start=True, stop=True)
            bc = sb.tile([128, NS], BF16, tag=f"bc{c}", name=f"bc{c}")
            nc.scalar.activation(bc[:], pbc[:], AF.Copy)
            binv_cs.append(bc)

        # ================= similarity tiles =================
        # For each 128-protein tile mi, each stationary operand ppb[:,k,mi]
        # serves 4 molecule blocks; the two 4-bank groups ping-pong so one
        # group's eviction overlaps the other's accumulation.
        for mi in range(MS // 128):
            for g in range(2):
                pts = [ps.tile([128, 512], F32, tag="mm", name=f"spt{_i}") for _i in range(4)]
                if mi == 0 and g == 0:
                    # first tile group: consume gathered blocks in arrival
                    # order (c0 fully first) so the post-AllGather block
                    # staircase doesn't stall the PE
                    for ci in range(4):
                        for k in range(KD):
                            nc.tensor.matmul(
                                pts[ci][:], ppb[:, k, 0:128],
                                mn_cs[ci][:, k * NS:(k + 1) * NS],
                                start=(k == 0), stop=(k == KD - 1),
                            )
                else:
                    for k in range(KD):
                        lw = ppb[:, k, mi * 128:(mi + 1) * 128]
                        for ci in range(4):
                            nc.tensor.matmul(
                                pts[ci][:], lw,
                                mn_cs[g * 4 + ci][:, k * NS:(k + 1) * NS],
                                start=(k == 0), stop=(k == KD - 1),
                            )
                for ci in range(4):
                    stile = sb.tile([128, NS], F32, tag="st", bufs=4)
                    # (psum * protein_scale) * molecule_inv_norms in one DVE op
                    nc.vector.scalar_tensor_tensor(
                        stile[:], pts[ci][:], scale_col[:, mi:mi + 1],
                        binv_cs[g * 4 + ci][:],
                        mybir.AluOpType.mult, mybir.AluOpType.mult)
                    dma_eng = nc.gpsimd if ci % 2 == 0 else nc.sync
                    dma_eng.dma_start(out=S[g * 4 + ci, mi], in_=stile[:])

    nc.compile()
    _CACHE["nc"] = nc
    return nc


def _tile_w(W):
    """W [Dout, K] (fp32) -> bf16 [128, DC, K] where element (p, h, k*128+m) =
    W[h*128+m, k*128+p]: whole matrix is one partition-major linear load and
    slice [:, h, k*128:(k+1)*128] is a ready-to-use stationary operand."""
    Dout, K = W.shape
    kc = K // 128
    t = W.reshape(DC, 128, kc, 128).transpose(3, 0, 2, 1)   # [p, h, k, m]
    return np.ascontiguousarray(t.reshape(128, DC, kc * 128).astype(BF16NP))


def _tile_x(Xshard):
    """X [rows, K] -> bf16 [128, KC, rows] feature-major partition-tiled."""
    rows, K = Xshard.shape
    kc = K // 128
    t = Xshard.reshape(rows, kc, 128).transpose(2, 1, 0)    # [p, k, rows]
    return np.ascontiguousarray(t.astype(BF16NP))


def kernel(molecule, protein, Wm1, bm1, Wm2, bm2, Wp1, bp1, Wp2, bp2,
           temperature):
    nc = _build()

    molecule = np.asarray(molecule, np.float32)
    protein = np.asarray(protein, np.float32)
    wm1 = _tile_w(np.asarray(Wm1, np.float32))
    wm2 = _tile_w(np.asarray(Wm2, np.float32))
    wp1 = _tile_w(np.asarray(Wp1, np.float32))
    wp2 = _tile_w(np.asarray(Wp2, np.float32))

    def tile_b(b):
        return np.asarray(b, np.float32).reshape(DC, 128).T

    biases = np.ascontiguousarray(np.stack(
        [tile_b(bm1), tile_b(bm2), tile_b(bp1), tile_b(bp2)], axis=1))
    invt = (1.0 / np.asarray(temperature, np.float32)).reshape(1, 1)

    in_maps = []
    for c in range(N_CORES):
        in_maps.append({
            "molT": _tile_x(molecule[c * NS:(c + 1) * NS]),
            "protT": _tile_x(protein[c * MS:(c + 1) * MS]),
            "wm1": wm1, "wm2": wm2, "wp1": wp1, "wp2": wp2,
            "biases": biases, "invtemp": invt,
        })

    _CACHE["in_maps"] = in_maps
    res = run_bass_kernel_spmd(nc, in_maps, list(range(N_CORES)))
    out = np.empty((M, N), np.float32)
    for c in range(N_CORES):
        # S block layout [c2, mi, 128, 512] -> rows mi*128+i, cols c2*512+j
        blk = res.results[c]["S"]                      # [8, 8, 128, 512]
        out[c * MS:(c + 1) * MS] = blk.transpose(1, 2, 0, 3).reshape(MS, N)
    return out
